# revision 4
# baseline (speedup 1.0000x reference)
"""Self-contained Bass/Trainium2 kernel for single-head causal self-attention.

reference semantics (fp32):
  qkv = x @ Wqkv; q,k,v = split(qkv)
  att = softmax(mask(q k^T / sqrt(C)))
  y = (att @ v) @ Wproj

Sharding: 8 cores = 4 batches x 2 sequence halves (2048 q-rows each).
Lower-half cores place their 2048 real rows at virtual positions
2048..4096 behind a masked junk prefix so all cores run one NEFF.
"""

import sys

sys.path.insert(0, "/opt/trn_rl_repo")

import numpy as np

B, T, C = 4, 4096, 512
TQ = 2048              # q rows per core
N_CORES = 8
NG = 4                 # q groups of 512 rows per core
SCALE = 1.0 / np.sqrt(C)
MASKVAL = -1.0e10

_CACHE = {}


def _dmask_np():
    # [128, 4*512] additive masks for the 4 diagonal-offset variants.
    # Variant d, sub-tile k columns: k<d fully masked, k==d triangular
    # (valid where j' <= i'), k>d fully visible.
    m = np.zeros((128, 4, 4, 128), dtype=np.float32)
    jj = np.arange(128)[:, None]
    ii = np.arange(128)[None, :]
    tri = np.where(jj <= ii, 0.0, MASKVAL).astype(np.float32)
    for d in range(4):
        for k in range(4):
            if k < d:
                m[:, d, k, :] = MASKVAL
            elif k == d:
                m[:, d, k, :] = tri
    return m.reshape(128, 4 * 512)


def _build():
    import concourse.mybir as mybir
    import concourse.tile as tile
    from concourse import bacc

    F32 = mybir.dt.float32
    F32R = mybir.dt.float32r
    AF = mybir.ActivationFunctionType

    nc = bacc.Bacc("TRN2", target_bir_lowering=False, debug=False,
                   num_devices=N_CORES)

    x_in = nc.dram_tensor("x_in", [T, C], F32, kind="ExternalInput").ap()
    m_in = nc.dram_tensor("m_in", [128, T // 128], F32, kind="ExternalInput").ap()
    wqkv_in = nc.dram_tensor("wqkv", [C, 3 * C], F32, kind="ExternalInput").ap()
    wproj_in = nc.dram_tensor("wproj", [C, C], F32, kind="ExternalInput").ap()
    y_out = nc.dram_tensor("y", [TQ, C], F32, kind="ExternalOutput").ap()
    v_scr = nc.dram_tensor("v_scr", [T, C], F32R, kind="Internal").ap()

    dmask_d = nc.inline_tensor(_dmask_np(), name="dmask").ap()
    ident_d = nc.inline_tensor(np.eye(128, dtype=np.float32), name="ident").ap()

    with tile.TileContext(nc) as tc:
        with tc.tile_pool(name="persist", bufs=1) as pp:
            kT = pp.tile([128, 4, T], F32R)          # K^T  [c-chunk, j]
            qT = pp.tile([128, 4, TQ], F32R)         # Q^T  [c-chunk, i]
            wproj_sb = pp.tile([128, 4, C], F32R)
            m_sb = pp.tile([128, T // 128], F32)     # per-j-chunk bias masks
            dm_sb = pp.tile([128, 4, 512], F32)      # diagonal masks
            id_sb = pp.tile([128, 128], F32)
            ones_r = pp.tile([128, 2], F32R)

            nc.sync.dma_start(id_sb[:], ident_d[:])

            # ---------------- Phase 1: x^T, K^T, Q^T, V ----------------
            with tc.tile_pool(name="wq", bufs=1) as wq_pool:
                wqkv_sb = wq_pool.tile([128, 4, 3 * C], F32R)
                with tc.tile_pool(name="wqtmp", bufs=1) as wqt:
                    wq_raw = wqt.tile([128, 4, 3 * C], F32)
                    nc.sync.dma_start(
                        wq_raw[:], wqkv_in.rearrange("(k p) f -> p k f", p=128))
                    nc.vector.tensor_copy(wqkv_sb[:], wq_raw[:])

                with tc.tile_pool(name="p1", bufs=3) as p1, \
                     tc.tile_pool(name="p1ps", bufs=2, space="PSUM") as p1ps:
                    for tch in range(T // 512):
                        x_t = p1.tile([128, 4, 512], F32, tag="x")
                        nc.sync.dma_start(
                            x_t[:],
                            x_in[512 * tch:512 * (tch + 1), :]
                            .rearrange("(n p) c -> p n c", p=128))
                        xT = p1.tile([128, 4, 512], F32R, tag="xT")
                        for n in range(4):
                            ps_xt = p1ps.tile([128, 512], F32, tag="xt")
                            for c in range(4):
                                nc.tensor.transpose(
                                    ps_xt[:, 128 * c:128 * (c + 1)],
                                    x_t[:, n, 128 * c:128 * (c + 1)],
                                    id_sb[:])
                            nc.vector.tensor_copy(
                                xT[:, :, 128 * n:128 * (n + 1)],
                                ps_xt[:].rearrange("p (c q) -> p c q", c=4))
                        # K^T tiles (and Q^T for the upper half)
                        for f in range(4):
                            ps_k = p1ps.tile([128, 512], F32, tag="kf")
                            for c in range(4):
                                nc.tensor.matmul(
                                    ps_k[:],
                                    wqkv_sb[:, c, C + 128 * f:C + 128 * (f + 1)],
                                    xT[:, c, :],
                                    start=(c == 0), stop=(c == 3))
                            nc.scalar.copy(
                                kT[:, f, 512 * tch:512 * (tch + 1)], ps_k[:])
                        if tch >= 4:
                            for f in range(4):
                                ps_q = p1ps.tile([128, 512], F32, tag="kf")
                                for c in range(4):
                                    nc.tensor.matmul(
                                        ps_q[:],
                                        wqkv_sb[:, c, 128 * f:128 * (f + 1)],
                                        xT[:, c, :],
                                        start=(c == 0), stop=(c == 3))
                                nc.scalar.copy(
                                    qT[:, f, 512 * (tch - 4):512 * (tch - 3)],
                                    ps_q[:])
                        # V tiles -> DRAM scratch
                        for n in range(4):
                            ps_v = p1ps.tile([128, 512], F32, tag="v")
                            for c in range(4):
                                nc.tensor.matmul(
                                    ps_v[:],
                                    xT[:, c, 128 * n:128 * (n + 1)],
                                    wqkv_sb[:, c, 2 * C:3 * C],
                                    start=(c == 0), stop=(c == 3))
                            v_sb = p1.tile([128, 512], F32R, tag="vsb")
                            nc.vector.tensor_copy(v_sb[:], ps_v[:])
                            r0 = 512 * tch + 128 * n
                            nc.sync.dma_start(v_scr[r0:r0 + 128, :], v_sb[:])

            # ---------------- Phase 2 constants ----------------
            nc.sync.dma_start(m_sb[:], m_in[:])
            nc.sync.dma_start(dm_sb[:], dmask_d.rearrange("p (d n) -> p d n", d=4))
            with tc.tile_pool(name="wtmp", bufs=1) as wt:
                ones_f = wt.tile([128, 2], F32)
                nc.vector.memset(ones_f[:], 1.0)
                nc.vector.tensor_copy(ones_r[:], ones_f[:])
                wp_raw = wt.tile([128, 4, C], F32)
                nc.sync.dma_start(wp_raw[:],
                                  wproj_in.rearrange("(k p) f -> p k f", p=128))
                nc.vector.tensor_copy(wproj_sb[:], wp_raw[:])

            # ---------------- Phase 2: attention + projection ----------------
            with tc.tile_pool(name="p2", bufs=1) as p2, \
                 tc.tile_pool(name="psS", bufs=2, space="PSUM") as psS, \
                 tc.tile_pool(name="psO", bufs=1, space="PSUM") as psO, \
                 tc.tile_pool(name="psl", bufs=1, space="PSUM") as psl, \
                 tc.tile_pool(name="psot", bufs=1, space="PSUM") as psot:
                for g in range(NG):
                    trip = 20 + 4 * g
                    o_ps = [psO.tile([128, 512], F32, tag=f"o{k}", name=f"o_ps{k}")
                            for k in range(4)]
                    l_ps = psl.tile([2, 512], F32, tag="l")
                    for t in range(trip):
                        v_t = p2.tile([128, 512], F32R, tag="vt", bufs=4)
                        nc.sync.dma_start(v_t[:], v_scr[128 * t:128 * (t + 1), :])
                        s_ps = psS.tile([128, 512], F32, tag="s")
                        for c in range(4):
                            nc.tensor.matmul(
                                s_ps[:],
                                kT[:, c, 128 * t:128 * (t + 1)],
                                qT[:, c, 512 * g:512 * (g + 1)],
                                start=(c == 0), stop=(c == 3))
                        d = t - (16 + 4 * g)
                        if d >= 0:
                            nc.vector.tensor_add(s_ps[:], s_ps[:], dm_sb[:, d, :])
                        pT = p2.tile([128, 512], F32R, tag="pT", bufs=3)
                        nc.scalar.activation(pT[:], s_ps[:], AF.Exp,
                                             bias=m_sb[:, t:t + 1], scale=SCALE)
                        first, last = (t == 0), (t == trip - 1)
                        nc.tensor.matmul(l_ps[:], ones_r[:], pT[:],
                                         start=first, stop=last)
                        for k in range(4):
                            nc.tensor.matmul(
                                o_ps[k][:], pT[:, 128 * k:128 * (k + 1)], v_t[:],
                                start=first, stop=last)
                    l_sb = p2.tile([2, 512], F32, tag="lsb", bufs=2)
                    nc.vector.tensor_copy(l_sb[:], l_ps[:])
                    lt_ps = psS.tile([128, 8], F32, tag="s", name="lt_ps")
                    for k in range(4):
                        nc.tensor.transpose(
                            lt_ps[:, 2 * k:2 * (k + 1)],
                            l_sb[:, 128 * k:128 * (k + 1)],
                            id_sb[0:2, 0:2])
                    for k in range(4):
                        r_sb = p2.tile([128, 1], F32, tag="r", bufs=2)
                        nc.vector.reciprocal(r_sb[:], lt_ps[:, 2 * k:2 * k + 1])
                        o_sb = p2.tile([128, 512], F32, tag="osb", bufs=2)
                        nc.vector.tensor_scalar_mul(o_sb[:], o_ps[k][:], r_sb[:])
                        ot_ps = psot.tile([128, 512], F32, tag="ot")
                        for c in range(4):
                            nc.tensor.transpose(
                                ot_ps[:, 128 * c:128 * (c + 1)],
                                o_sb[:, 128 * c:128 * (c + 1)], id_sb[:])
                        oT_sb = p2.tile([128, 512], F32R, tag="oT", bufs=2)
                        nc.vector.tensor_copy(oT_sb[:], ot_ps[:])
                        y_ps = psS.tile([128, 512], F32, tag="s")
                        for c in range(4):
                            nc.tensor.matmul(
                                y_ps[:], oT_sb[:, 128 * c:128 * (c + 1)],
                                wproj_sb[:, c, :],
                                start=(c == 0), stop=(c == 3))
                        y_sb = p2.tile([128, 512], F32, tag="ysb", bufs=2)
                        nc.scalar.copy(y_sb[:], y_ps[:])
                        r0 = 128 * (4 * g + k)
                        nc.sync.dma_start(y_out[r0:r0 + 128, :], y_sb[:])
    nc.compile()
    return nc


def _get_nc():
    if "nc" not in _CACHE:
        _CACHE["nc"] = _build()
    return _CACHE["nc"]


def kernel(x, Wqkv, Wproj, _trace=False):
    from concourse.bass_utils import run_bass_kernel_spmd

    x = np.ascontiguousarray(x, dtype=np.float32)
    Wqkv = np.ascontiguousarray(Wqkv, dtype=np.float32)
    Wproj = np.ascontiguousarray(Wproj, dtype=np.float32)

    m_lo_flat = np.concatenate([
        np.full(TQ, MASKVAL, dtype=np.float32),
        np.zeros(TQ, dtype=np.float32)])
    m_lo = np.ascontiguousarray(m_lo_flat.reshape(T // 128, 128).T)
    m_hi = np.zeros((128, T // 128), dtype=np.float32)
    zpad = np.zeros((TQ, C), dtype=np.float32)

    in_maps = []
    for core in range(N_CORES):
        b, h = core // 2, core % 2
        if h == 0:
            x_v = np.concatenate([zpad, x[b, :TQ]], axis=0)
            m_v = m_lo
        else:
            x_v = x[b]
            m_v = m_hi
        in_maps.append({
            "x_in": np.ascontiguousarray(x_v),
            "m_in": m_v,
            "wqkv": Wqkv,
            "wproj": Wproj,
        })

    nc = _get_nc()
    kw = {"trace": True} if _trace else {}
    br = run_bass_kernel_spmd(nc, in_maps, core_ids=list(range(N_CORES)), **kw)

    out = np.empty((B, T, C), dtype=np.float32)
    for core in range(N_CORES):
        b, h = core // 2, core % 2
        out[b, h * TQ:(h + 1) * TQ] = br.results[core]["y"]
    if _trace:
        _CACHE["last_results"] = br
    return out


# revision 6
# speedup vs baseline: 1.1073x; 1.1073x over previous
"""Self-contained Bass/Trainium2 kernel for single-head causal self-attention.

reference semantics (fp32):
  qkv = x @ Wqkv; q,k,v = split(qkv)
  att = softmax(mask(q k^T / sqrt(C)))
  y = (att @ v) @ Wproj

Sharding: 8 cores = 4 batches x 2 sequence halves (2048 q-rows each).
Lower-half cores place their 2048 real rows at virtual positions
2048..4096 behind a masked junk prefix so all cores run one NEFF.
"""

import sys

sys.path.insert(0, "/opt/trn_rl_repo")

import numpy as np

B, T, C = 4, 4096, 512
TQ = 2048              # q rows per core
N_CORES = 8
NG = 4                 # q groups of 512 rows per core
SCALE = 1.0 / np.sqrt(C)
MASKVAL = -1.0e10

_CACHE = {}


def _dmask_np():
    # [128, 4*512] additive masks for the 4 diagonal-offset variants.
    # Variant d, sub-tile k columns: k<d fully masked, k==d triangular
    # (valid where j' <= i'), k>d fully visible.
    m = np.zeros((128, 4, 4, 128), dtype=np.float32)
    jj = np.arange(128)[:, None]
    ii = np.arange(128)[None, :]
    tri = np.where(jj <= ii, 0.0, MASKVAL).astype(np.float32)
    for d in range(4):
        for k in range(4):
            if k < d:
                m[:, d, k, :] = MASKVAL
            elif k == d:
                m[:, d, k, :] = tri
    return m.reshape(128, 4 * 512)


def _build():
    import concourse.mybir as mybir
    import concourse.tile as tile
    from concourse import bacc

    F32 = mybir.dt.float32
    F32R = mybir.dt.float32r
    AF = mybir.ActivationFunctionType

    nc = bacc.Bacc("TRN2", target_bir_lowering=False, debug=False,
                   num_devices=N_CORES)

    x_in = nc.dram_tensor("x_in", [T, C], F32, kind="ExternalInput").ap()
    m_in = nc.dram_tensor("m_in", [128, T // 128], F32, kind="ExternalInput").ap()
    wqkv_in = nc.dram_tensor("wqkv", [C, 3 * C], F32, kind="ExternalInput").ap()
    wproj_in = nc.dram_tensor("wproj", [C, C], F32, kind="ExternalInput").ap()
    y_out = nc.dram_tensor("y", [TQ, C], F32, kind="ExternalOutput").ap()
    v_scr = nc.dram_tensor("v_scr", [T, C], F32R, kind="Internal").ap()

    dmask_d = nc.inline_tensor(_dmask_np(), name="dmask").ap()
    ident_d = nc.inline_tensor(np.eye(128, dtype=np.float32), name="ident").ap()

    with tile.TileContext(nc) as tc:
        with tc.tile_pool(name="persist", bufs=1) as pp:
            kT = pp.tile([128, 4, T], F32R)          # K^T  [c-chunk, j]
            qT = pp.tile([128, 4, TQ], F32R)         # Q^T  [c-chunk, i]
            wproj_sb = pp.tile([128, 4, C], F32R)
            m_sb = pp.tile([128, T // 128], F32)     # per-j-chunk bias masks
            dm_sb = pp.tile([128, 4, 512], F32)      # diagonal masks
            id_sb = pp.tile([128, 128], F32)
            ones_r = pp.tile([128, 2], F32R)

            nc.sync.dma_start(id_sb[:], ident_d[:])

            # ---------------- Phase 1: x^T, K^T, Q^T, V ----------------
            with tc.tile_pool(name="wq", bufs=1) as wq_pool:
                wqkv_sb = wq_pool.tile([128, 4, 3 * C], F32R)
                with tc.tile_pool(name="p1", bufs=3) as p1, \
                     tc.tile_pool(name="wqtmp", bufs=1) as wqt, \
                     tc.tile_pool(name="p1ps", bufs=2, space="PSUM") as p1ps:
                    x_tiles = {}
                    for tch in range(2):
                        x_pre = p1.tile([128, 4, 512], F32, tag="x",
                                        name=f"x_pre{tch}")
                        nc.sync.dma_start(
                            x_pre[:],
                            x_in[512 * tch:512 * (tch + 1), :]
                            .rearrange("(n p) c -> p n c", p=128))
                        x_tiles[tch] = x_pre
                    for c in range(4):
                        wq_raw = wqt.tile([128, 3 * C], F32, tag="wqr", bufs=2)
                        nc.sync.dma_start(
                            wq_raw[:],
                            wqkv_in[128 * c:128 * (c + 1), :])
                        nc.vector.tensor_copy(wqkv_sb[:, c, :], wq_raw[:])

                    for tch in range(T // 512):
                        if tch in x_tiles:
                            x_t = x_tiles[tch]
                        else:
                            x_t = p1.tile([128, 4, 512], F32, tag="x")
                            nc.sync.dma_start(
                                x_t[:],
                                x_in[512 * tch:512 * (tch + 1), :]
                                .rearrange("(n p) c -> p n c", p=128))
                        xT = p1.tile([128, 4, 512], F32R, tag="xT")
                        for n in range(4):
                            ps_xt = p1ps.tile([128, 512], F32, tag="xt")
                            for c in range(4):
                                nc.tensor.transpose(
                                    ps_xt[:, 128 * c:128 * (c + 1)],
                                    x_t[:, n, 128 * c:128 * (c + 1)],
                                    id_sb[:])
                            nc.vector.tensor_copy(
                                xT[:, :, 128 * n:128 * (n + 1)],
                                ps_xt[:].rearrange("p (c q) -> p c q", c=4))
                        # K^T tiles (and Q^T for the upper half)
                        for f in range(4):
                            ps_k = p1ps.tile([128, 512], F32, tag="kf")
                            for c in range(4):
                                nc.tensor.matmul(
                                    ps_k[:],
                                    wqkv_sb[:, c, C + 128 * f:C + 128 * (f + 1)],
                                    xT[:, c, :],
                                    start=(c == 0), stop=(c == 3))
                            nc.scalar.copy(
                                kT[:, f, 512 * tch:512 * (tch + 1)], ps_k[:])
                        if tch >= 4:
                            for f in range(4):
                                ps_q = p1ps.tile([128, 512], F32, tag="kf")
                                for c in range(4):
                                    nc.tensor.matmul(
                                        ps_q[:],
                                        wqkv_sb[:, c, 128 * f:128 * (f + 1)],
                                        xT[:, c, :],
                                        start=(c == 0), stop=(c == 3))
                                nc.scalar.copy(
                                    qT[:, f, 512 * (tch - 4):512 * (tch - 3)],
                                    ps_q[:])
                        # V tiles -> DRAM scratch
                        for n in range(4):
                            ps_v = p1ps.tile([128, 512], F32, tag="v")
                            for c in range(4):
                                nc.tensor.matmul(
                                    ps_v[:],
                                    xT[:, c, 128 * n:128 * (n + 1)],
                                    wqkv_sb[:, c, 2 * C:3 * C],
                                    start=(c == 0), stop=(c == 3))
                            v_sb = p1.tile([128, 512], F32R, tag="vsb")
                            nc.vector.tensor_copy(v_sb[:], ps_v[:])
                            r0 = 512 * tch + 128 * n
                            nc.sync.dma_start(v_scr[r0:r0 + 128, :], v_sb[:])

            # ---------------- Phase 2 constants ----------------
            nc.sync.dma_start(m_sb[:], m_in[:])
            nc.sync.dma_start(dm_sb[:], dmask_d.rearrange("p (d n) -> p d n", d=4))
            with tc.tile_pool(name="wtmp", bufs=1) as wt:
                ones_f = wt.tile([128, 2], F32)
                nc.vector.memset(ones_f[:], 1.0)
                nc.vector.tensor_copy(ones_r[:], ones_f[:])
                wp_raw = wt.tile([128, 4, C], F32)
                nc.sync.dma_start(wp_raw[:],
                                  wproj_in.rearrange("(k p) f -> p k f", p=128))
                nc.vector.tensor_copy(wproj_sb[:], wp_raw[:])

            # ---------------- Phase 2: attention + projection ----------------
            with tc.tile_pool(name="p2", bufs=1) as p2, \
                 tc.tile_pool(name="psS", bufs=2, space="PSUM") as psS, \
                 tc.tile_pool(name="psO", bufs=1, space="PSUM") as psO, \
                 tc.tile_pool(name="psl", bufs=1, space="PSUM") as psl, \
                 tc.tile_pool(name="psot", bufs=1, space="PSUM") as psot:
                for g in range(NG):
                    trip = 20 + 4 * g
                    o_ps = [psO.tile([128, 512], F32, tag=f"o{k}", name=f"o_ps{k}")
                            for k in range(4)]
                    l_ps = psl.tile([128, 8], F32, tag="l")
                    for t in range(trip):
                        v_t = p2.tile([128, 512], F32R, tag="vt", bufs=4)
                        nc.sync.dma_start(v_t[:], v_scr[128 * t:128 * (t + 1), :])
                        s_ps = psS.tile([128, 512], F32, tag="s")
                        for c in range(4):
                            nc.tensor.matmul(
                                s_ps[:],
                                kT[:, c, 128 * t:128 * (t + 1)],
                                qT[:, c, 512 * g:512 * (g + 1)],
                                start=(c == 0), stop=(c == 3))
                        d = t - (16 + 4 * g)
                        if d >= 0:
                            nc.vector.tensor_add(s_ps[:], s_ps[:], dm_sb[:, d, :])
                        pT = p2.tile([128, 512], F32R, tag="pT", bufs=3)
                        nc.scalar.activation(pT[:], s_ps[:], AF.Exp,
                                             bias=m_sb[:, t:t + 1], scale=SCALE)
                        first, last = (t == 0), (t == trip - 1)
                        for k in range(4):
                            nc.tensor.matmul(
                                o_ps[k][:], pT[:, 128 * k:128 * (k + 1)], v_t[:],
                                start=first, stop=last)
                            nc.tensor.matmul(
                                l_ps[:, 2 * k:2 * (k + 1)],
                                pT[:, 128 * k:128 * (k + 1)], ones_r[:],
                                start=(first and k == 0), stop=last,
                                skip_group_check=True)
                    for k in range(4):
                        r_sb = p2.tile([128, 1], F32, tag="r", bufs=2)
                        nc.vector.reciprocal(r_sb[:], l_ps[:, 2 * k:2 * k + 1])
                        o_sb = p2.tile([128, 512], F32, tag="osb", bufs=2)
                        nc.vector.tensor_scalar_mul(o_sb[:], o_ps[k][:], r_sb[:])
                        ot_ps = psot.tile([128, 512], F32, tag="ot")
                        for c in range(4):
                            nc.tensor.transpose(
                                ot_ps[:, 128 * c:128 * (c + 1)],
                                o_sb[:, 128 * c:128 * (c + 1)], id_sb[:])
                        oT_sb = p2.tile([128, 512], F32R, tag="oT", bufs=2)
                        nc.vector.tensor_copy(oT_sb[:], ot_ps[:])
                        y_ps = psS.tile([128, 512], F32, tag="s")
                        for c in range(4):
                            nc.tensor.matmul(
                                y_ps[:], oT_sb[:, 128 * c:128 * (c + 1)],
                                wproj_sb[:, c, :],
                                start=(c == 0), stop=(c == 3))
                        y_sb = p2.tile([128, 512], F32, tag="ysb", bufs=2)
                        nc.scalar.copy(y_sb[:], y_ps[:])
                        r0 = 128 * (4 * g + k)
                        nc.sync.dma_start(y_out[r0:r0 + 128, :], y_sb[:])
    nc.compile()
    return nc


def _get_nc():
    if "nc" not in _CACHE:
        _CACHE["nc"] = _build()
    return _CACHE["nc"]


def kernel(x, Wqkv, Wproj, _trace=False):
    from concourse.bass_utils import run_bass_kernel_spmd

    x = np.ascontiguousarray(x, dtype=np.float32)
    Wqkv = np.ascontiguousarray(Wqkv, dtype=np.float32)
    Wproj = np.ascontiguousarray(Wproj, dtype=np.float32)

    m_lo_flat = np.concatenate([
        np.full(TQ, MASKVAL, dtype=np.float32),
        np.zeros(TQ, dtype=np.float32)])
    m_lo = np.ascontiguousarray(m_lo_flat.reshape(T // 128, 128).T)
    m_hi = np.zeros((128, T // 128), dtype=np.float32)
    zpad = np.zeros((TQ, C), dtype=np.float32)

    in_maps = []
    for core in range(N_CORES):
        b, h = core // 2, core % 2
        if h == 0:
            x_v = np.concatenate([zpad, x[b, :TQ]], axis=0)
            m_v = m_lo
        else:
            x_v = x[b]
            m_v = m_hi
        in_maps.append({
            "x_in": np.ascontiguousarray(x_v),
            "m_in": m_v,
            "wqkv": Wqkv,
            "wproj": Wproj,
        })

    nc = _get_nc()
    kw = {"trace": True} if _trace else {}
    br = run_bass_kernel_spmd(nc, in_maps, core_ids=list(range(N_CORES)), **kw)

    out = np.empty((B, T, C), dtype=np.float32)
    for core in range(N_CORES):
        b, h = core // 2, core % 2
        out[b, h * TQ:(h + 1) * TQ] = br.results[core]["y"]
    if _trace:
        _CACHE["last_results"] = br
    return out
